# revision 18
# baseline (speedup 1.0000x reference)
"""Trainium2 Bass kernel: Conv3d(3,24,k=3,VALID) -> min over depth -> softmax over channels.

Input  x: [16,3,32,128,128] f32, conv_weight [24,3,3,3,3], conv_bias [24].
Output: [16,24,1,126,126] f32.

v3 strategy (per core; batch-sharded 2 samples/core over 8 cores):
 - 26 h-blocks of 5 output rows (hbase = 5b); 7 pages x 4 blocks.
 - 4 concurrent 32-row PE tiles (positions 32i); rows k = ci*7 + hl
   (21 used), M = 120 cols = hoff*24 + co (5 hoff x 24 co).
 - (kd,kw) = 9 PSUM accumulation passes with shifted rhs APs; fp16
   operands (PE streams ~1 row/cycle with >=192B contiguous w runs).
 - PSUM chunks (d5,w96)x6 + (d15,w32)x2 per block: long moving-fetch runs.
 - Depth-min (weights negated -> max): DVE tensor_reduce straight from
   PSUM into fp16 max accumulators + fp16 combines.
 - Softmax: ACT exp(bias-max) -> PE transpose -> DVE sum/recip/bcast-mult.
"""
import sys

sys.path.insert(0, "/opt/trn_rl_repo")

import numpy as np

# Problem constants
N_TOT, CI, D, H, W = 16, 3, 32, 128, 128
CO = 24
DO, HO, WO = 30, 126, 126
NCORES = 8
NPC = N_TOT // NCORES  # samples per core = 2

MM_DT = "float16"

NBLK = 26          # h-blocks of 5 output rows
NPAGE = 7          # 4 blocks per page
DP, WP = 32, 130   # padded per-partition (d, w) extents
FREE = NPC * DP * WP
# (wbase, wlen, dg, d0) single-bank PSUM chunk descriptors
CHUNKS = [(0, 96, 5, 0), (0, 96, 5, 5), (0, 96, 5, 10), (0, 96, 5, 15),
          (0, 96, 5, 20), (0, 96, 5, 25), (96, 32, 15, 0), (96, 32, 15, 15)]

_cache = {}


def _np_mmdt():
    if MM_DT in ("float32", "float32r"):
        return np.float32
    if MM_DT == "float16":
        return np.float16
    import ml_dtypes
    return ml_dtypes.bfloat16


def _build_program():
    import concourse.bass as bass
    import concourse.mybir as mybir
    from concourse import bacc, tile

    dt = mybir.dt
    mdt = getattr(dt, MM_DT)
    f32 = dt.float32
    f16 = dt.float16
    AX = mybir.AxisListType
    ALU = mybir.AluOpType
    ACT_F = mybir.ActivationFunctionType

    nc = bacc.Bacc("TRN2", target_bir_lowering=False, debug=False)

    xs = nc.dram_tensor("xs", [NPAGE, 128, FREE], mdt, kind="ExternalInput")
    wt = nc.dram_tensor("wt", [128, 9 * 128], mdt, kind="ExternalInput")
    bias = nc.dram_tensor("bias", [128, 1], f32, kind="ExternalInput")
    ident = nc.dram_tensor("ident", [128, 128], f32, kind="ExternalInput")
    # [n, ho, wo, co] so output DMA descriptors are 96B-contiguous runs
    out = nc.dram_tensor("out", [NPC, HO, WO, CO], f32, kind="ExternalOutput")

    with tile.TileContext(nc) as tc:
        with (
            tc.tile_pool(name="const", bufs=1) as constp,
            tc.tile_pool(name="xpage", bufs=2) as xpagep,
            tc.tile_pool(name="expp", bufs=4) as expp,
            tc.tile_pool(name="soft", bufs=4) as softp,
            tc.tile_pool(name="ps", bufs=2, space="PSUM") as psp,
        ):
            wt_t = constp.tile([128, 9 * 128], mdt)
            nc.sync.dma_start(wt_t[:], wt[:])
            bias_t = constp.tile([128, 1], f32)
            nc.sync.dma_start(bias_t[:], bias[:])
            ident_t = constp.tile([128, 128], f32)
            nc.sync.dma_start(ident_t[:], ident[:])

            ov = out[:].rearrange("n h w c -> n w h c")
            pending = []  # lagged epilogue closures

            for P in range(NPAGE):
                page_t = xpagep.tile([128, FREE], mdt)
                pv = page_t[:].rearrange("p (n d w) -> p n d w",
                                         n=NPC, d=DP, w=WP)
                nc.sync.dma_start(page_t[:], xs[P])
                nblk = min(4, NBLK - 4 * P)  # blocks on this page

                for n in range(NPC):
                    exp16s = [expp.tile([128, 128], f16, tag=f"exp{i}",
                                        name=f"exp16_{i}")
                              for i in range(nblk)]
                    tmpAs = [softp.tile([128, 576], f16, tag=f"tmpa{i}",
                                        name=f"tmpa_{i}")
                             for i in range(nblk)]
                    tmpBs = [softp.tile([128, 64], f16, tag=f"tmpb{i}",
                                        name=f"tmpb_{i}")
                             for i in range(nblk)]
                    if nblk == 4:
                        slots = [[(i, i, c) for i in range(4)]
                                 for c in range(8)]
                    else:
                        # last page: 2 blocks mirrored into the upper 64
                        # partitions; each block's 8 chunks split across a
                        # position pair so all 4 PE positions stay busy
                        slots = [[(0, 0, c), (1, 1, c),
                                  (2, 0, 4 + c), (3, 1, 4 + c)]
                                 for c in range(4)]
                    for slot in slots:
                        convs = {}
                        for pos, bl, c in slot:
                            convs[pos] = psp.tile(
                                [128, 512], f32, tag=f"conv{pos}",
                                name=f"conv_{pos}")
                        for p9 in range(9):
                            kd, kw = p9 // 3, p9 % 3
                            for pos, bl, c in slot:
                                wbase, wlen, dg, d0 = CHUNKS[c]
                                ovw = convs[pos][:, 0:480].rearrange(
                                    "p (d w) -> p d w", w=wlen)
                                nc.tensor.matmul(
                                    ovw,
                                    lhsT=wt_t[32 * pos:32 * pos + 32,
                                              128 * p9:128 * p9 + 128],
                                    rhs=pv[32 * pos:32 * pos + 32, n,
                                           d0 + kd:d0 + kd + dg,
                                           wbase + kw:wbase + kw + wlen],
                                    start=(p9 == 0),
                                    stop=(p9 == 8),
                                    tile_position=(32 * pos, 0),
                                )
                        for pos, bl, c in slot:
                            wbase, wlen, dg, d0 = CHUNKS[c]
                            gi = d0 // dg
                            rin = convs[pos][:, 0:480].rearrange(
                                "p (d w) -> p w d", w=wlen)
                            dst = (tmpAs[bl][:, 96 * gi:96 * gi + 96]
                                   if wbase == 0 else
                                   tmpBs[bl][:, 32 * gi:32 * gi + 32])
                            nc.vector.tensor_reduce(
                                dst, rin, axis=AX.X, op=ALU.max)

                    for i in range(nblk):
                        # fold the 6+2 depth-group partials in two ops
                        nc.vector.tensor_reduce(
                            exp16s[i][:, 0:96],
                            tmpAs[i][:].rearrange("p (g w) -> p w g", g=6),
                            axis=AX.X, op=ALU.max)
                        nc.vector.tensor_reduce(
                            exp16s[i][:, 96:128],
                            tmpBs[i][:].rearrange("p (g w) -> p w g", g=2),
                            axis=AX.X, op=ALU.max)

                    exp_outs = []
                    for i in range(nblk):
                        exp_out = expp.tile([128, 128], f32, tag=f"expo{i}",
                                            name=f"expo_{i}")
                        nc.scalar.activation(
                            exp_out[:], exp16s[i][:], ACT_F.Exp,
                            bias=bias_t[:, 0:1], scale=-1.0)
                        exp_outs.append(exp_out)

                    def ep2(P=P, n=n, nblk=nblk, exp_outs=exp_outs):
                        for i in range(nblk):
                            b = 4 * P + i
                            nho = min(5, HO - 5 * b)
                            tp = psp.tile([128, 512], f32, tag="conv0")
                            nc.tensor.matmul(
                                tp[:, 0:128], lhsT=exp_outs[i][:],
                                rhs=ident_t[:], is_transpose=True,
                                start=True, stop=True)
                            tpx = tp[:, 0:120].rearrange(
                                "p (h c) -> p h c", h=5)
                            sums = softp.tile([128, 16], f32, tag="sums")
                            nc.vector.tensor_reduce(
                                sums[:, 0:5], tpx, axis=AX.X, op=ALU.add)
                            nc.vector.reciprocal(
                                sums[:, 8:13], sums[:, 0:5])
                            ost = softp.tile([128, 120], f32, tag="ost")
                            ostv = ost[:].rearrange("p (h c) -> p h c", h=5)
                            rec = sums[:, 8:13].unsqueeze(-1) \
                                .broadcast_to([128, 5, 24])
                            nc.vector.tensor_tensor(
                                ostv, tpx, rec, op=ALU.mult)
                            nc.sync.dma_start(
                                ov[n, :, 5 * b:5 * b + nho],
                                ost[0:WO, 0:24 * nho].rearrange(
                                    "p (h c) -> p h c", h=nho))

                    pending.append(ep2)
                    if len(pending) > 1:
                        pending.pop(0)()
            for fn in pending:
                fn()
    nc.compile()
    return nc


def _prep_tables(conv_weight, conv_bias):
    Wn = -np.asarray(conv_weight, np.float32)  # negate: min -> max
    # wt[p9 = kd*3+kw][row k = ci*7 + hl][col = hoff*24 + co]
    wt = np.zeros((9, 32, 128), np.float32)
    for p9 in range(9):
        kd, kw = p9 // 3, p9 % 3
        for ci in range(CI):
            for hl in range(7):
                k = ci * 7 + hl
                for hoff in range(5):
                    kh = hl - hoff
                    if 0 <= kh < 3:
                        wt[p9, k, hoff * 24 + np.arange(CO)] = \
                            Wn[:, ci, kd, kh, kw]
    wt_flat = wt.transpose(1, 0, 2).reshape(32, 9 * 128)
    wt128 = np.tile(wt_flat, (4, 1)).astype(_np_mmdt())

    bias = np.zeros((128, 1), np.float32)
    b = np.asarray(conv_bias, np.float32)
    for hoff in range(5):
        bias[hoff * 24:hoff * 24 + 24, 0] = b
    return wt128, bias


def _block_x(xc):
    """[NPC,3,32,128,128] -> [NPAGE, 128, NPC*32*130] pre-blocked pages."""
    xc = np.asarray(xc, np.float32)
    blk = np.zeros((NPAGE, 128, NPC, DP, WP), np.float32)
    for P in range(NPAGE):
        for i in range(min(4, NBLK - 4 * P)):
            hbase = 5 * (4 * P + i)
            for ci in range(CI):
                for hl in range(7):
                    h = hbase + hl
                    if h >= H:
                        continue
                    part = 32 * i + ci * 7 + hl
                    blk[P, part, :, 0:D, 0:W] = xc[:, ci, :, h, :]
    # mirror the last page's 2 blocks into the upper 64 partitions so all
    # four PE tile positions stay busy (chunks split across position pairs)
    blk[NPAGE - 1, 64:128] = blk[NPAGE - 1, 0:64]
    return blk.reshape(NPAGE, 128, FREE).astype(_np_mmdt())


def _get_runner():
    """Build the bass program and a cached jitted SPMD executor once."""
    if "runner" in _cache:
        return _cache["runner"]
    import jax
    from jax.experimental.shard_map import shard_map
    from jax.sharding import Mesh, PartitionSpec
    from concourse import bass2jax

    nc = _build_program()
    _cache["nc"] = nc
    bass2jax.install_neuronx_cc_hook()

    import concourse.mybir as mybir

    pname = nc.partition_id_tensor.name if nc.partition_id_tensor else None
    in_names, out_names, out_avals, zero_outs = [], [], [], []
    for alloc in nc.m.functions[0].allocations:
        if not isinstance(alloc, mybir.MemoryLocationSet):
            continue
        name = alloc.memorylocations[0].name
        if alloc.kind == "ExternalInput":
            if name != pname:
                in_names.append(name)
        elif alloc.kind == "ExternalOutput":
            out_names.append(name)
            shape = tuple(alloc.tensor_shape)
            dtype = mybir.dt.np(alloc.dtype)
            out_avals.append(jax.core.ShapedArray(shape, dtype))
            zero_outs.append(np.zeros(shape, dtype))
    n_params = len(in_names)
    n_outs = len(out_avals)
    all_names = in_names + out_names + ([pname] if pname else [])

    def _body(*args):
        operands = list(args)
        if pname:
            operands.append(bass2jax.partition_id_tensor())
        outs = bass2jax._bass_exec_p.bind(
            *operands,
            out_avals=tuple(out_avals),
            in_names=tuple(all_names),
            out_names=tuple(out_names),
            lowering_input_output_aliases=(),
            sim_require_finite=True,
            sim_require_nnan=True,
            nc=nc,
        )
        return tuple(outs)

    devices = jax.devices()[:NCORES]
    mesh = Mesh(np.asarray(devices), ("core",))
    in_specs = (PartitionSpec("core"),) * (n_params + n_outs)
    out_specs = (PartitionSpec("core"),) * n_outs
    donate = tuple(range(n_params, n_params + n_outs))
    sharded = jax.jit(
        shard_map(_body, mesh=mesh, in_specs=in_specs, out_specs=out_specs,
                  check_rep=False),
        donate_argnums=donate, keep_unused=True)

    def run(in_maps):
        per_core = [[np.asarray(m[name]) for name in in_names]
                    for m in in_maps]
        concat_in = [
            np.concatenate([per_core[c][i] for c in range(NCORES)], axis=0)
            for i in range(n_params)
        ]
        concat_zeros = [
            np.zeros((NCORES * z.shape[0], *z.shape[1:]), z.dtype)
            for z in zero_outs
        ]
        out_arrs = sharded(*concat_in, *concat_zeros)
        return [
            {name: np.asarray(out_arrs[i]).reshape(
                NCORES, *out_avals[i].shape)[c]
             for i, name in enumerate(out_names)}
            for c in range(NCORES)
        ]

    _cache["runner"] = run
    return run


def kernel(x, conv_weight, conv_bias):
    x = np.asarray(x, np.float32)
    wt128, bias = _prep_tables(conv_weight, conv_bias)
    ident = np.eye(128, dtype=np.float32)

    run = _get_runner()
    in_maps = [
        {
            "xs": _block_x(x[NPC * c:NPC * (c + 1)]),
            "wt": wt128,
            "bias": bias,
            "ident": ident,
        }
        for c in range(NCORES)
    ]
    results = run(in_maps)
    outs = [results[c]["out"] for c in range(NCORES)]
    full = np.concatenate(outs, axis=0)  # [16,126,126,24] (n,ho,wo,co)
    full = np.ascontiguousarray(full.transpose(0, 3, 1, 2))
    return full.reshape(N_TOT, CO, 1, HO, WO).astype(np.float32)
